# revision 1
# baseline (speedup 1.0000x reference)
"""Trainium2 Bass kernel for nn_GumbelLinear (topk_masking).

Computation:
  h (64,16) -> conditional range-remap (global min/max of h) ->
  mask = h @ w_p + bias -> logits = mask + g1 - g2 (Gumbel noise from
  U1/U2) -> per-row top-5 hard mask (straight-through).

Sharding: replicate h (needed for the global min/max) and w_p; data-parallel
the 64-row axis across 8 cores (8 rows each).  Host side only reshapes /
transposes / slices / concatenates numpy arrays; all math runs on device.

Device notes:
  - All per-core inputs are packed host-side into ONE [16,136] f32 tensor so
    a single DMA brings everything in (six separate DMAs serialize on the
    sync queue and cost ~600ns each to issue).
  - h is consumed transposed (hT [16,64]) so the contraction dim lands on
    partitions for the PE matmul.
  - Global max/-min: DVE-only — per-partition X-reduce into two columns of a
    -1e30-filled [32,32] block, 32x32 stream-transpose, one X-reduce over
    both rows, then two stream-shuffle broadcasts (partition 0/1 -> all).
  - sigmoid is strictly monotonic, so the top-5 threshold compare runs on
    logits directly; the hard straight-through output is the 0/1 mask itself
    (reference's (hard_bin - soft) + soft equals hard_bin to 1 ulp).
    This also kills the second ACT table load (Ln and Sigmoid live in
    different tables; each load costs ~1.3us).
  - A dependency-free dummy Ln on the eps tile pulls the single ACT table
    load to kernel start, overlapping the input DMA.
"""

import numpy as np

N_CORES = 8
ROWS = 64
D = 16
RPC = ROWS // N_CORES  # rows per core
EPS = 1e-8

# packed layout: tensor A [16, 88] (h-side, needed first on the critical
# path), tensor B [8, 48] (bias/U1/U2, consumed later by the ACT engine)
C_HT = 0       # [0:16, 0:64]   h transposed (full, replicated)
C_HTS = 64     # [0:16, 64:72]  this core's 8 rows of h, transposed
C_WP = 72      # [0:16, 72:88]  w_p
CA_END = 88
C_BIAS = 0     # [0:8, 0:16]    bias rows
C_U1 = 16      # [0:8, 16:32]   U1 rows (flattened)
C_U2 = 32      # [0:8, 32:48]   U2 rows (flattened)
CB_END = 48

_CACHE = {}


def _build_nc():
    import concourse.tile as tile
    from concourse import bacc, mybir

    f32 = mybir.dt.float32
    Alu = mybir.AluOpType
    Act = mybir.ActivationFunctionType

    nc = bacc.Bacc("TRN2", debug=False, enable_asserts=False)

    packed_a = nc.dram_tensor("packed_a", (D, CA_END), f32, kind="ExternalInput")
    packed_b = nc.dram_tensor("packed_b", (RPC, CB_END), f32, kind="ExternalInput")
    out_s = nc.dram_tensor("out_s", (RPC, D), f32, kind="ExternalOutput")

    with tile.TileContext(nc) as tc:
        with (
            tc.tile_pool(name="sb", bufs=1) as sb,
            tc.tile_pool(name="ps", bufs=1, space=tile.bass.MemorySpace.PSUM) as ps,
        ):
            t = sb.tile([D, CA_END], f32)
            nc.sync.dma_start(t[:], packed_a[:])
            tb = sb.tile([RPC, CB_END], f32)
            nc.sync.dma_start(tb[:], packed_b[:])
            v_hT = t[:, C_HT:C_HTS]
            v_hTs = t[:, C_HTS:C_WP]
            v_wp = t[:, C_WP:CA_END]
            v_bias = tb[:, C_BIAS:C_U1]
            v_u1 = tb[:, C_U1:C_U2]
            v_u2 = tb[:, C_U2:CB_END]

            # dep-free setup: eps tile + dummy Ln (pulls the ACT table load
            # to kernel start, overlapping the input DMA)
            eps_t = sb.tile([RPC, 1], f32)
            nc.vector.memset(eps_t[:], EPS)
            dscr = sb.tile([1, 1], f32)
            nc.scalar.activation(
                dscr[:], eps_t[0:1, 0:1], Act.Ln, bias=eps_t[0:1, :], scale=1.0
            )

            # ---- global max / -min of h, broadcast to all partitions ----
            NEG = -1.0e30
            scr = sb.tile([32, 33], f32)
            nc.vector.memset(scr[:], NEG)
            scrT = sb.tile([32, 33], f32)
            nc.vector.memset(scrT[:], NEG)
            nc.vector.tensor_reduce(
                scr[0:D, 0:1], v_hT, axis=mybir.AxisListType.X, op=Alu.max
            )
            nc.vector.tensor_reduce(
                scr[0:D, 1:2], v_hT, axis=mybir.AxisListType.X, op=Alu.min,
                negate=True,
            )
            nc.vector.transpose(scrT[:, 0:32], scr[:, 0:32])
            # scrT row 0 = per-column maxes, row 1 = negated per-column mins
            nc.vector.tensor_reduce(
                scrT[0:2, 32:33], scrT[0:2, 0:32], axis=mybir.AxisListType.X,
                op=Alu.max,
            )
            bc = sb.tile([32, 2], f32)
            nc.vector.stream_shuffle(bc[:, 0:1], scrT[:, 32:33], mask=[0] * 32)
            nc.vector.stream_shuffle(bc[:, 1:2], scrT[:, 32:33], mask=[1] * 32)
            gmax = bc[0:D, 0:1]  # max(h) on every partition
            mneg = bc[0:D, 1:2]  # -min(h) on every partition

            # s = 1.0 if out-of-range else 0.0
            tmx = sb.tile([D, 1], f32)
            nc.vector.tensor_max(tmx[:], gmax, mneg)
            s = sb.tile([D, 1], f32)
            nc.vector.tensor_scalar(s[:], tmx[:], 100.0, None, op0=Alu.is_gt)

            # mapped = clip((h - min)/(max - min)*0.6 - 0.3, -.3, .3)
            # rcp6 = 0.6/(max-min) via rng06 = (gmax+mneg)/0.6
            rng06 = sb.tile([D, 1], f32)
            nc.vector.tensor_scalar(
                rng06[:], gmax, mneg, 1.0 / 0.6, op0=Alu.add, op1=Alu.mult
            )
            rcp6 = sb.tile([D, 1], f32)
            nc.vector.reciprocal(rcp6[:], rng06[:])
            m0 = sb.tile([D, RPC], f32)
            nc.vector.tensor_scalar(
                m0[:], v_hTs, mneg, rcp6[:], op0=Alu.add, op1=Alu.mult
            )
            m1 = sb.tile([D, RPC], f32)
            nc.vector.tensor_scalar(
                m1[:], m0[:], 0.3, -0.3, op0=Alu.subtract, op1=Alu.max
            )
            # dlt = clip(m1) - h;  hu = h + s*dlt
            dlt = sb.tile([D, RPC], f32)
            nc.vector.scalar_tensor_tensor(
                dlt[:], in0=m1[:], scalar=0.3, in1=v_hTs,
                op0=Alu.min, op1=Alu.subtract,
            )
            hu = sb.tile([D, RPC], f32)
            i_hu = nc.vector.scalar_tensor_tensor(
                hu[:], in0=dlt[:], scalar=s[:], in1=v_hTs,
                op0=Alu.mult, op1=Alu.add,
            )

            # ---- matmul: pm[RPC, D] = hu.T @ wp ----
            pm = ps.tile([RPC, D], f32)
            nc.tensor.matmul(pm[:], hu[:], v_wp, start=True, stop=True)

            # ---- Gumbel: b = ln(-ln(U + eps) + eps); g = -b (ACT) ----
            a1 = sb.tile([RPC, D], f32)
            nc.scalar.activation(a1[:], v_u1, Act.Ln, bias=eps_t[:], scale=1.0)
            b1 = sb.tile([RPC, D], f32)
            nc.scalar.activation(b1[:], a1[:], Act.Ln, bias=eps_t[:], scale=-1.0)
            a2 = sb.tile([RPC, D], f32)
            nc.scalar.activation(a2[:], v_u2, Act.Ln, bias=eps_t[:], scale=1.0)
            b2 = sb.tile([RPC, D], f32)
            nc.scalar.activation(b2[:], a2[:], Act.Ln, bias=eps_t[:], scale=-1.0)

            # base = bias + g1 - g2 = bias - b1 + b2.  Ordered after `hu`
            # (nosync dep) so these don't interleave into the middle of the
            # critical DVE chain — they fill the bubble during the matmul.
            from concourse.tile_rust import add_dep_helper

            gg = sb.tile([RPC, D], f32)
            i_gg = nc.vector.tensor_sub(gg[:], b2[:], b1[:])
            add_dep_helper(i_gg.ins, i_hu.ins, sync=False)
            base = sb.tile([RPC, D], f32)
            nc.vector.tensor_add(base[:], gg[:], v_bias)

            # logits = mask + base; sigmoid is monotonic so the top-5
            # threshold compare runs on logits directly
            logits = sb.tile([RPC, D], f32)
            nc.vector.tensor_add(logits[:], pm[:], base[:])
            top8 = sb.tile([RPC, 8], f32)
            nc.vector.max(top8[:], logits[:])
            hard = sb.tile([RPC, D], f32)
            nc.vector.tensor_scalar(
                hard[:], logits[:], top8[:, 4:5], None, op0=Alu.is_ge
            )

            nc.sync.dma_start(out_s[:], hard[:])

    nc.compile()
    return nc


def _get_nc():
    if "nc" not in _CACHE:
        _CACHE["nc"] = _build_nc()
    return _CACHE["nc"]


def _make_in_maps(h, w_p, bias, U1, U2):
    h = np.ascontiguousarray(np.asarray(h, np.float32).reshape(ROWS, D))
    hT = h.T
    wp = np.asarray(w_p, np.float32)
    bias = np.asarray(bias, np.float32).reshape(ROWS, D)
    u1 = np.asarray(U1, np.float32).reshape(ROWS, D)
    u2 = np.asarray(U2, np.float32).reshape(ROWS, D)

    in_maps = []
    for c in range(N_CORES):
        rows = slice(c * RPC, (c + 1) * RPC)
        pa = np.empty((D, CA_END), np.float32)
        pa[:, C_HT:C_HTS] = hT
        pa[:, C_HTS:C_WP] = h[rows].T
        pa[:, C_WP:CA_END] = wp
        pb = np.empty((RPC, CB_END), np.float32)
        pb[:, C_BIAS:C_U1] = bias[rows]
        pb[:, C_U1:C_U2] = u1[rows]
        pb[:, C_U2:CB_END] = u2[rows]
        in_maps.append({"packed_a": pa, "packed_b": pb})
    return in_maps


def kernel(h, input, w_p, bias, U1, U2, **_unused):
    from concourse.bass_utils import run_bass_kernel_spmd

    nc = _get_nc()
    in_maps = _make_in_maps(h, w_p, bias, U1, U2)
    res = run_bass_kernel_spmd(nc, in_maps, core_ids=list(range(N_CORES)))
    out = np.concatenate([r["out_s"] for r in res.results], axis=0)
    return out.reshape(ROWS, 4, 4).astype(np.float32)



# revision 7
# speedup vs baseline: 1.1264x; 1.1264x over previous
"""Trainium2 Bass kernel for nn_GumbelLinear (topk_masking).

Computation (reference):
  h (64,16) -> conditional range-remap (global min/max of h) ->
  mask = h @ w_p + bias -> logits = mask + g1 - g2 (Gumbel noise from
  U1/U2) -> per-row top-5 hard mask (straight-through).

Reformulation used here: the range-remap's min/max are GLOBAL scalars, so
remap(h) = A*h + B' with scalars A = s*(rcp6-1)+1, and the remapped matmul
folds into  logits = A*(h@w_p) + B*colsum(w_p) + bias + g1 - g2  where
B = s*(mneg*rcp6 - 0.3).  The matmul therefore starts the moment the input
DMA lands (no dependency on the min/max chain), and colsum(w_p) comes free
from an augmented ones-column in the stationary operand.

Sharding: replicate h (global min/max) and w_p; data-parallel the 64-row
axis across 8 cores (8 rows each).

Implementation notes:
  - Raw Bass (no TileContext): hand-placed semaphores, no tile-exit
    barrier/RANGE_CLEAR block (~0.8us), fewer per-op waits.
  - ONE input DMA per core: everything host-packed into [16,144] including
    eps column and the ones column (zero device-side memsets on the
    critical path; the framework's const memsets aside).
  - Global max / -min: per-partition X-reduce writes columns 89/90 of the
    input tile, 32x32 stream-transpose over cols 89:121, second X-reduce
    restricted to cols 0:16 (cols 16-31 of the transposed rows come from
    uninitialized partitions 16-31 and are never read), then two
    stream-shuffle broadcasts.
  - Gumbel: U1|U2 processed as one [8,32] Ln pair on ACT (2 activations
    instead of 4); ACT table load is hoisted to stream start by the
    compiler and overlaps the input DMA.
  - sigmoid is monotonic, so the top-5 threshold compare runs on logits
    directly; the straight-through output equals the 0/1 mask.
  - STRIP_PREAMBLE removes the framework's dead const-AP memsets and the
    redundant init all-engine barrier (the NEFF runtime prefix already
    rendezvous-gates all engines before the kernel body).
"""

import numpy as np

N_CORES = 8
ROWS = 64
D = 16
RPC = ROWS // N_CORES  # rows per core
EPS = 1e-8
NEG = -1.0e30

# packed [16, 144] layout (columns)
C_HT = 0       # [0:16,   0:64]  h transposed (full, replicated)
C_OWN = 64     # [0:16,  64:72]  this core's 8 rows of h, transposed
C_ONE = 72     # [0:16,  72:73]  ones (colsum(w_p) via augmented matmul)
C_WP = 73      # [0:16,  73:89]  w_p
C_MAX = 89     # [0:16,  89:90]  reduce dst: per-partition max of h
C_MIN = 90     # [0:16,  90:91]  reduce dst: per-partition -min of h
C_BIAS = 91    # [0:8,  91:107]  bias rows
C_U = 107      # [0:8, 107:139]  U1 | U2 rows (flattened)
C_EPS = 139    # [0:8, 139:140]  eps
C_END = 144

STRIP_PREAMBLE = True

_CACHE = {}


def _strip_framework_preamble(nc, preamble_names):
    """Drop the dead const-AP memsets and the init all-engine barrier the
    framework emits before our first instruction.  The NEFF runtime prefix
    already gates every engine behind a rendezvous, so the extra barrier
    only delays the input DMA (~1.3us on the measured critical path)."""
    from concourse import mybir

    kill = (mybir.InstMemset, mybir.InstDrain, mybir.InstEventSemaphore)
    for func in nc.m.functions:
        for block in func.blocks:
            keep = [
                i
                for i in block.instructions
                if not (i.name in preamble_names and isinstance(i, kill))
            ]
            if len(keep) != len(block.instructions):
                block.instructions = keep


def _build_nc():
    from concourse import bacc, mybir

    f32 = mybir.dt.float32
    Alu = mybir.AluOpType
    Act = mybir.ActivationFunctionType
    X = mybir.AxisListType.X

    nc = bacc.Bacc("TRN2", debug=False, enable_asserts=False)

    m_dram = nc.dram_tensor("packed_m", (D, C_END), f32, kind="ExternalInput")
    out_s = nc.dram_tensor("out_s", (RPC, D), f32, kind="ExternalOutput")

    preamble_names = {
        i.name for f in nc.m.functions for b in f.blocks for i in b.instructions
    }

    M = nc.alloc_sbuf_tensor("M", [32, C_END], f32)
    scrT = nc.alloc_sbuf_tensor("scrT", [32, 33], f32)
    bc = nc.alloc_sbuf_tensor("bc", [32, 2], f32)
    a12 = nc.alloc_sbuf_tensor("a12", [RPC, 32], f32)
    b12 = nc.alloc_sbuf_tensor("b12", [RPC, 32], f32)
    gg = nc.alloc_sbuf_tensor("gg", [RPC, D], f32)
    base = nc.alloc_sbuf_tensor("base", [RPC, D], f32)
    sc = nc.alloc_sbuf_tensor("sc", [RPC, 8], f32)
    bcs = nc.alloc_sbuf_tensor("bcs", [32, D], f32)
    t1 = nc.alloc_sbuf_tensor("t1", [RPC, D], f32)
    lg = nc.alloc_sbuf_tensor("lg", [RPC, D], f32)
    top8 = nc.alloc_sbuf_tensor("top8", [RPC, 8], f32)
    hard = nc.alloc_sbuf_tensor("hard", [RPC, D], f32)
    P = nc.alloc_psum_tensor("P", [32, D], f32)

    # Engines have NO intra-engine write->read hazard interlock: a
    # dependent op on the same engine must wait for the producer's
    # @complete semaphore (this is exactly what Tile's per-op sem chains
    # do).  One counting semaphore per engine; every producer incs it at
    # write-retire, every consumer (same- or cross-engine) waits on the
    # producer's count.
    sd = nc.alloc_semaphore("sd")      # input DMA landed
    smm = nc.alloc_semaphore("smm")    # matmul done (PSUM ready)
    aq = nc.alloc_semaphore("aq")      # ACT op counter
    pq = nc.alloc_semaphore("pq")      # Pool op counter
    dq = nc.alloc_semaphore("dq")      # DVE op counter
    so = nc.alloc_semaphore("so")      # output DMA landed

    # eps column view used as the ACT bias pointer
    v_eps = M[0:RPC, C_EPS : C_EPS + 1]

    # ---- Sync: input DMA; output DMA at the end ----
    nc.sync.dma_start(M[0:D, :], m_dram[:, :]).then_inc(sd, 16)

    # ---- PE: pm_aug = [hT_own | 1]^T @ w_p -> P[0:9]; row 8 = colsum(wp)
    nc.tensor.wait_ge(sd, 16)
    nc.tensor.matmul(
        P[0 : RPC + 1, :],
        M[0:D, C_OWN : C_ONE + 1],
        M[0:D, C_WP:C_MAX],
        start=True,
        stop=True,
    ).then_inc(smm, 1)

    # ---- ACT: b12 = ln(-ln(U12 + eps) + eps)  (= -g for each half) ----
    nc.scalar.wait_ge(sd, 16)
    nc.scalar.activation(
        a12[:, :], M[0:RPC, C_U : C_U + 32], Act.Ln, bias=v_eps, scale=1.0
    ).then_inc(aq, 1)
    nc.scalar.wait_ge(aq, 1)
    nc.scalar.activation(
        b12[:, :], a12[:, :], Act.Ln, bias=v_eps, scale=-1.0
    ).then_inc(aq, 1)

    # ---- GpSimd: gumbel join; scalar tail A/B ----
    # pq counts: 1 gg, 2 base, 3 tmx, 4 s, 5 tA, 6 A, 7 tB, 8 B
    nc.gpsimd.wait_ge(aq, 2)
    nc.gpsimd.tensor_sub(gg[:, :], b12[:, D : 2 * D], b12[:, 0:D]).then_inc(pq, 1)
    nc.gpsimd.wait_ge(pq, 1)
    nc.gpsimd.tensor_add(
        base[:, :], gg[:, :], M[0:RPC, C_BIAS : C_BIAS + D]
    ).then_inc(pq, 1)
    nc.gpsimd.wait_ge(dq, 6)
    # tmx = max(gmax, mneg); s = tmx > 100
    nc.gpsimd.tensor_scalar(
        sc[:, 0:1], bc[0:RPC, 0:1], bc[0:RPC, 1:2], None, op0=Alu.max
    ).then_inc(pq, 1)
    nc.gpsimd.wait_ge(pq, 3)
    nc.gpsimd.tensor_scalar(
        sc[:, 1:2], sc[:, 0:1], 100.0, None, op0=Alu.is_gt
    ).then_inc(pq, 1)
    # A = s*(rcp6 - 1) + 1
    nc.gpsimd.wait_ge(dq, 8)
    nc.gpsimd.wait_ge(pq, 4)
    nc.gpsimd.tensor_scalar(
        sc[:, 4:5], sc[:, 3:4], 1.0, sc[:, 1:2], op0=Alu.subtract, op1=Alu.mult
    ).then_inc(pq, 1)
    nc.gpsimd.wait_ge(pq, 5)
    nc.gpsimd.tensor_scalar(sc[:, 5:6], sc[:, 4:5], 1.0, None, op0=Alu.add).then_inc(
        pq, 1
    )
    # B = s*(mneg*rcp6 - 0.3)
    nc.gpsimd.tensor_scalar(
        sc[:, 6:7], bc[0:RPC, 1:2], sc[:, 3:4], None, op0=Alu.mult
    ).then_inc(pq, 1)
    nc.gpsimd.wait_ge(pq, 7)
    nc.gpsimd.tensor_scalar(
        sc[:, 7:8], sc[:, 6:7], 0.3, sc[:, 1:2], op0=Alu.subtract, op1=Alu.mult
    ).then_inc(pq, 1)

    # ---- DVE: global max chain, broadcast, logits, top-5 mask ----
    # dq counts: 1 rmax, 2 rmin, 3 transpose, 4 reduce2, 5 shuf0, 6 shuf1,
    #            7 rng06, 8 rcp6, 9 bcs, 10 t1, 11 lg, 12 top8, 13 hard
    nc.vector.wait_ge(sd, 16)
    nc.vector.tensor_reduce(
        M[0:D, C_MAX : C_MAX + 1], M[0:D, C_HT:C_OWN], axis=X, op=Alu.max
    ).then_inc(dq, 1)
    nc.vector.tensor_reduce(
        M[0:D, C_MIN : C_MIN + 1], M[0:D, C_HT:C_OWN], axis=X, op=Alu.min,
        negate=True,
    ).then_inc(dq, 1)
    nc.vector.wait_ge(dq, 2)
    nc.vector.transpose(scrT[:, 0:32], M[0:32, C_MAX : C_MAX + 32]).then_inc(dq, 1)
    nc.vector.wait_ge(dq, 3)
    nc.vector.tensor_reduce(
        scrT[0:2, 32:33], scrT[0:2, 0:D], axis=X, op=Alu.max
    ).then_inc(dq, 1)
    nc.vector.wait_ge(dq, 4)
    nc.vector.stream_shuffle(bc[:, 0:1], scrT[:, 32:33], mask=[0] * 32).then_inc(
        dq, 1
    )
    nc.vector.stream_shuffle(bc[:, 1:2], scrT[:, 32:33], mask=[1] * 32).then_inc(
        dq, 1
    )
    # rng06 = (gmax + mneg)/0.6 ; rcp6 = 1/rng06
    nc.vector.wait_ge(dq, 6)
    nc.vector.tensor_scalar(
        sc[:, 2:3], bc[0:RPC, 0:1], bc[0:RPC, 1:2], 1.0 / 0.6,
        op0=Alu.add, op1=Alu.mult,
    ).then_inc(dq, 1)
    nc.vector.wait_ge(dq, 7)
    nc.vector.reciprocal(sc[:, 3:4], sc[:, 2:3]).then_inc(dq, 1)
    nc.vector.wait_ge(smm, 1)
    nc.vector.stream_shuffle(bcs[:, :], P[0:32, :], mask=[RPC] * 32).then_inc(dq, 1)
    # t1 = A*pm + base ; lg = B*cs + t1
    nc.vector.wait_ge(pq, 6)
    nc.vector.scalar_tensor_tensor(
        t1[:, :], in0=P[0:RPC, :], scalar=sc[:, 5:6], in1=base[:, :],
        op0=Alu.mult, op1=Alu.add,
    ).then_inc(dq, 1)
    nc.vector.wait_ge(pq, 8)
    nc.vector.wait_ge(dq, 10)
    nc.vector.scalar_tensor_tensor(
        lg[:, :], in0=bcs[0:RPC, :], scalar=sc[:, 7:8], in1=t1[:, :],
        op0=Alu.mult, op1=Alu.add,
    ).then_inc(dq, 1)
    nc.vector.wait_ge(dq, 11)
    nc.vector.max(top8[:, :], lg[:, :]).then_inc(dq, 1)
    nc.vector.wait_ge(dq, 12)
    nc.vector.tensor_scalar(
        hard[:, :], lg[:, :], top8[:, 4:5], None, op0=Alu.is_ge
    ).then_inc(dq, 1)

    # ---- Sync: output DMA + completion fence ----
    nc.sync.wait_ge(dq, 13)
    nc.sync.dma_start(out_s[:, :], hard[:, :]).then_inc(so, 16)
    nc.sync.wait_ge(so, 16)

    if STRIP_PREAMBLE:
        _strip_framework_preamble(nc, preamble_names)

    nc.compile()
    return nc


def _get_nc():
    if "nc" not in _CACHE:
        _CACHE["nc"] = _build_nc()
    return _CACHE["nc"]


def _make_in_maps(h, w_p, bias, U1, U2):
    h = np.ascontiguousarray(np.asarray(h, np.float32).reshape(ROWS, D))
    hT = h.T
    wp = np.asarray(w_p, np.float32)
    bias = np.asarray(bias, np.float32).reshape(ROWS, D)
    u1 = np.asarray(U1, np.float32).reshape(ROWS, D)
    u2 = np.asarray(U2, np.float32).reshape(ROWS, D)

    in_maps = []
    for c in range(N_CORES):
        rows = slice(c * RPC, (c + 1) * RPC)
        m = np.zeros((D, C_END), np.float32)
        m[:, C_HT:C_OWN] = hT
        m[:, C_OWN:C_ONE] = h[rows].T
        m[:, C_ONE] = 1.0
        m[:, C_WP:C_MAX] = wp
        m[:, C_MAX : C_MIN + 1] = NEG
        m[0:RPC, C_BIAS : C_BIAS + D] = bias[rows]
        m[0:RPC, C_U : C_U + D] = u1[rows]
        m[0:RPC, C_U + D : C_U + 2 * D] = u2[rows]
        m[0:RPC, C_EPS] = EPS
        in_maps.append({"packed_m": m})
    return in_maps


def kernel(h, input, w_p, bias, U1, U2, **_unused):
    from concourse.bass_utils import run_bass_kernel_spmd

    nc = _get_nc()
    in_maps = _make_in_maps(h, w_p, bias, U1, U2)
    res = run_bass_kernel_spmd(nc, in_maps, core_ids=list(range(N_CORES)))
    out = np.concatenate([r["out_s"] for r in res.results], axis=0)
    return out.reshape(ROWS, 4, 4).astype(np.float32)
